# revision 26
# baseline (speedup 1.0000x reference)
"""GNN message-passing kernel for Trainium2 (8 NeuronCores, Bass/Tile).

Pipeline (matches reference.py):
  MLP head (Linear -> BN(eval) -> ReLU -> Linear)        [N,128] -> [N,40]
  10 hops of nxt = segment_sum(norm * carry[src], dst)   sparse A @ carry
  sigmoid attention over the 11 hop snapshots, log_softmax.

Strategy:
  - Destinations sharded over 8 cores; nodes permuted host-side by degree
    (snake-dealt for balance, degree-sorted within shard so each 128-dst
    tile has near-uniform in-degree).
  - Per dst tile of 128 nodes: R_t "rounds"; round r slot p holds the r-th
    in-edge of dst (tile_base+p) (idx = permuted src, dummy idx 0/norm 0).
  - Per hop: indirect-DMA gather of 128 exact 80B rows per round from the
    all-gathered carry (fp16), in-place multiply by an SBUF-loaded
    expanded norm table, strided free-dim reduce over rounds -> nxt tile.
  - fp16 carry communicated with a 1/4 per-hop scale (values grow ~3.5x
    per hop and would overflow fp16); unscale factors are folded into the
    sigmoid-attention, which is accumulated incrementally per hop so hop
    snapshots never hit DRAM.
  - Per-hop AllGather (8 cores) of the fp16 carry shards.
"""
import sys
sys.path.insert(0, "/opt/trn_rl_repo")

import numpy as np
import concourse.bass as bass

N = 169343
F = 128
CLS = 40
HID = 256
KHOPS = 10
NCORES = 8
P = 128
N8 = 21248            # rows per core (128*166), padded
NT = N8 // P          # 166 dst tiles per core
NPAD = N8 * NCORES
N8H = 16384           # AllGather issued in two parts; ag layout is
NTH = 128             # (part, core, part-shard) so the big lo part can ship
                      # early and hide under the remaining gathers; only the
                      # small hi part (38 tiles, ~latency-floor) stays exposed
GMAX = 64             # max rounds per gather buffer
WG = 8                # tiles per write group
BN_EPS = 1e-5

_COMPILED = {}


# ----------------------------------------------------------------------------
# host-side preprocessing
# ----------------------------------------------------------------------------

def _prep(x, edge_index, norm, W1, b1, bn_gamma, bn_beta, bn_mean, bn_var,
          W2, b2, proj_w, proj_b):
    src = np.asarray(edge_index[0], dtype=np.int64)
    dst = np.asarray(edge_index[1], dtype=np.int64)
    E = src.shape[0]
    deg = np.bincount(dst, minlength=N)

    # snake-deal nodes (descending degree) to cores for edge balance
    order = np.argsort(-deg, kind="stable")
    blk = np.arange(N) // NCORES
    lane = np.arange(N) % NCORES
    core_of_rank = np.where(blk % 2 == 0, lane, NCORES - 1 - lane)
    pos_of_rank = blk
    newid = np.empty(N, dtype=np.int64)
    newid[order] = core_of_rank * N8 + pos_of_rank

    # per-tile round counts, shared across cores (max over cores)
    degs_new = np.zeros(NPAD, dtype=np.int64)
    degs_new[newid] = deg
    degs_new = degs_new.reshape(NCORES, NT, P)
    R_list = np.maximum(degs_new.max(axis=(0, 2)), 1).astype(np.int64)  # [NT]
    roff = np.concatenate([[0], np.cumsum(R_list)])
    RT = int(roff[-1])

    # pack edges: for edge e: nd=newid[dst], r = rank within its dst
    nd = newid[dst]
    order2 = np.argsort(nd, kind="stable")
    nd_s = nd[order2]
    src_s = newid[src[order2]]
    norm_s = np.asarray(norm, dtype=np.float32)[order2]
    counts = np.bincount(nd_s, minlength=NPAD)
    starts = np.concatenate([[0], np.cumsum(counts)])[:-1]
    r_in = np.arange(E, dtype=np.int64) - starts[nd_s]

    c_e = nd_s // N8
    pos_e = nd_s % N8
    t_e = pos_e // P
    slot_e = pos_e % P
    col_e = roff[t_e] + r_in

    # exact-row gather: idx selects the 80B carry row in the split-AG
    # layout: lo half rows [c*N8H + pos], hi half rows [NPAD/2 + c*N8H + ...]
    sc = src_s // N8
    sp = src_s % N8
    agrow = np.where(sp < N8H, sc * N8H + sp,
                     NCORES * N8H + sc * (N8 - N8H) + (sp - N8H))
    idxall = np.zeros((NCORES, P, RT), dtype=np.int32)
    normv = np.zeros((NCORES, P, RT), dtype=np.float16)
    idxall[c_e, slot_e, col_e] = agrow.astype(np.int32)
    normv[c_e, slot_e, col_e] = norm_s.astype(np.float16)
    normexp = np.repeat(normv[:, :, :, None], CLS, axis=3).reshape(
        NCORES, P, RT * CLS)

    # x: permute rows to new order, pad, transpose, fp16
    xT = np.zeros((NCORES, P, N8), dtype=np.float16)
    xp = np.asarray(x, dtype=np.float32)
    for c in range(NCORES):
        rows = np.zeros((N8, F), dtype=np.float32)
        mask_rank = core_of_rank == c
        orig_ids = order[mask_rank]
        rows[pos_of_rank[mask_rank]] = xp[orig_ids]
        xT[c] = rows.T.astype(np.float16)

    # folded BN constants
    A = (np.asarray(bn_gamma) / np.sqrt(np.asarray(bn_var) + BN_EPS)).astype(np.float32)
    B = ((np.asarray(b1) - np.asarray(bn_mean)) * A + np.asarray(bn_beta)).astype(np.float32)
    bnab = np.stack([A[:128], A[128:], B[:128], B[128:]], axis=1)  # [128, 4]

    w1t = np.asarray(W1, dtype=np.float16)                        # [128, 256]
    w2p = np.stack([np.asarray(W2[:128], dtype=np.float16),
                    np.asarray(W2[128:], dtype=np.float16)], axis=1)  # [128,2,40]
    w2p = w2p.reshape(P, 2 * CLS)
    b2c = np.asarray(b2, dtype=np.float32).reshape(CLS, 1)
    projw128 = np.tile(np.asarray(proj_w, dtype=np.float32)[None, :], (P, 1))
    pb = float(np.asarray(proj_b).reshape(-1)[0])

    in_maps = []
    for c in range(NCORES):
        in_maps.append({
            "xT": xT[c],
            "w1t": w1t,
            "w2p": w2p,
            "bnab": bnab.astype(np.float32),
            "b2c": b2c,
            "projw128": projw128,
            "idxall": idxall[c],
            "normexp": normexp[c],
        })
    meta = dict(R_list=tuple(int(r) for r in R_list), RT=RT, pb=pb,
                order=order, core_of_rank=core_of_rank, pos_of_rank=pos_of_rank)
    return in_maps, meta


# ----------------------------------------------------------------------------
# device program
# ----------------------------------------------------------------------------

def _build(R_list, RT, pb, nhops=KHOPS, do_ag=True):
    import concourse.bass as bass
    import concourse.bacc as bacc
    import concourse.mybir as mybir
    import concourse.tile as tile
    from concourse.masks import make_identity

    f16 = mybir.dt.float16
    f32 = mybir.dt.float32
    i32 = mybir.dt.int32
    ALU = mybir.AluOpType
    ACTF = mybir.ActivationFunctionType

    roff = [0]
    for r in R_list:
        roff.append(roff[-1] + r)

    # gather groups: consecutive tiles with sum(R) <= GMAX (optionally also
    # capping tiles per group, used for the last hop so the post-gather
    # attention+softmax tail of the final group stays short)
    def make_groups(max_tiles):
        gs, cur, cursum = [], [], 0
        for t in range(NT):
            if cur and (cursum + R_list[t] > GMAX or len(cur) >= max_tiles):
                gs.append(cur)
                cur, cursum = [], 0
            cur.append(t)
            cursum += R_list[t]
        if cur:
            gs.append(cur)
        return gs

    groups = make_groups(NT)
    groups_last = make_groups(10)

    nc = bacc.Bacc("TRN2", target_bir_lowering=False, debug=False,
                   num_devices=NCORES, num_swdge_queues=2)

    xT_d = nc.dram_tensor("xT", [P, N8], f16, kind="ExternalInput")
    w1t_d = nc.dram_tensor("w1t", [P, HID], f16, kind="ExternalInput")
    w2p_d = nc.dram_tensor("w2p", [P, 2 * CLS], f16, kind="ExternalInput")
    bnab_d = nc.dram_tensor("bnab", [P, 4], f32, kind="ExternalInput")
    b2c_d = nc.dram_tensor("b2c", [CLS, 1], f32, kind="ExternalInput")
    pw_d = nc.dram_tensor("projw128", [P, CLS], f32, kind="ExternalInput")
    idx_d = nc.dram_tensor("idxall", [P, RT], i32, kind="ExternalInput")
    nexp_d = nc.dram_tensor("normexp", [P, RT * CLS], f16, kind="ExternalInput")
    out_d = nc.dram_tensor("out", [N8, CLS], f32, kind="ExternalOutput")

    comm = [nc.dram_tensor(f"comm{k}", [N8, CLS], f16, kind="Internal")
            for k in range(KHOPS)]
    ag = [nc.dram_tensor(f"ag{k}", [NPAD, CLS], f16, kind="Internal")
          for k in range(KHOPS)]
    rgroups = [list(range(NCORES))]

    def emit_ag_lo(k):
        nc.gpsimd.collective_compute(
            "AllGather", mybir.AluOpType.bypass, replica_groups=rgroups,
            ins=[comm[k][0:N8H, :]], outs=[ag[k][0:NCORES * N8H, :]])

    def emit_ag_hi(k):
        nc.gpsimd.collective_compute(
            "AllGather", mybir.AluOpType.bypass, replica_groups=rgroups,
            ins=[comm[k][N8H:N8, :]], outs=[ag[k][NCORES * N8H:NPAD, :]])

    with tile.TileContext(nc) as tc:
        with tc.tile_pool(name="const", bufs=1) as cpool:
            idxt = cpool.tile([P, RT], i32)
            pw = cpool.tile([P, CLS], f32)
            w1s = cpool.tile([P, HID], f16)
            w2s = cpool.tile([P, 2 * CLS], f16)
            bnab = cpool.tile([P, 4], f32)
            b2s = cpool.tile([CLS, 1], f32)
            ident = cpool.tile([P, P], f32)
            acc = cpool.tile([P, NT * CLS], f32)
            hbuf = cpool.tile([P, NT * CLS], f32)
            nexp = cpool.tile([P, RT * CLS], f16)
            nc.sync.dma_start(out=idxt[:], in_=idx_d[:])
            nc.sync.dma_start(out=pw[:], in_=pw_d[:])
            nc.sync.dma_start(out=w1s[:], in_=w1t_d[:])
            nc.sync.dma_start(out=w2s[:], in_=w2p_d[:])
            nc.sync.dma_start(out=bnab[:], in_=bnab_d[:])
            nc.sync.dma_start(out=b2s[:], in_=b2c_d[:])
            nc.sync.dma_start(out=nexp[:], in_=nexp_d[:])
            make_identity(nc, ident[:])

            # ---------------- MLP phase ----------------
            with tc.tile_pool(name="mlp", bufs=2) as mpool, \
                 tc.tile_pool(name="psum", bufs=2, space="PSUM") as ppool:
                r0 = 0
                while r0 < N8:
                    rows = min(512, N8 - r0)
                    nchunk = rows // P
                    xt = mpool.tile([P, rows], f16, tag="xt")
                    nc.sync.dma_start(out=xt[:], in_=xT_d[:, r0:r0 + rows])
                    ph0 = ppool.tile([P, rows], f32, tag="ph0", space="PSUM")
                    ph1 = ppool.tile([P, rows], f32, tag="ph1", space="PSUM")
                    nc.tensor.matmul(out=ph0[:], lhsT=w1s[:, 0:P], rhs=xt[:],
                                     start=True, stop=True)
                    nc.tensor.matmul(out=ph1[:], lhsT=w1s[:, P:HID], rhs=xt[:],
                                     start=True, stop=True)
                    hs0 = mpool.tile([P, rows], f16, tag="hs0")
                    hs1 = mpool.tile([P, rows], f16, tag="hs1")
                    nc.scalar.activation(out=hs0[:], in_=ph0[:], func=ACTF.Relu,
                                         scale=bnab[:, 0:1], bias=bnab[:, 2:3])
                    nc.scalar.activation(out=hs1[:], in_=ph1[:], func=ACTF.Relu,
                                         scale=bnab[:, 1:2], bias=bnab[:, 3:4])
                    po = ppool.tile([CLS, rows], f32, tag="po", space="PSUM")
                    nc.tensor.matmul(out=po[:], lhsT=w2s[:, 0:CLS], rhs=hs0[:],
                                     start=True, stop=False)
                    nc.tensor.matmul(out=po[:], lhsT=w2s[:, CLS:2 * CLS],
                                     rhs=hs1[:], start=False, stop=True)
                    osb = mpool.tile([CLS, rows], f32, tag="osb")
                    nc.scalar.activation(out=osb[:], in_=po[:],
                                         func=ACTF.Identity, bias=b2s[:, 0:1])
                    wb = mpool.tile([P, nchunk * CLS], f16, tag="wb")
                    for j in range(nchunk):
                        t_glob = (r0 + j * P) // P
                        pt = ppool.tile([P, CLS], f32, tag="pt", space="PSUM")
                        nc.tensor.transpose(out=pt[:],
                                            in_=osb[:, j * P:(j + 1) * P],
                                            identity=ident[:CLS, :CLS])
                        nc.scalar.activation(out=wb[:, j * CLS:(j + 1) * CLS],
                                             in_=pt[:], func=ACTF.Copy)
                        nc.scalar.copy(
                            out=hbuf[:, t_glob * CLS:(t_glob + 1) * CLS],
                            in_=pt[:])
                    dst_ap = comm[0][r0:r0 + rows, :].rearrange(
                        "(g p) c -> p g c", p=P)
                    nc.sync.dma_start(out=dst_ap, in_=wb[:].rearrange(
                        "p (g c) -> p g c", c=CLS))
                    if (do_ag and nhops >= 1 and r0 < N8H <= r0 + rows):
                        emit_ag_lo(0)
                    r0 += rows

            if do_ag and nhops >= 1:
                emit_ag_hi(0)

            # deferred hop-0 attention: overlaps AG0 + hop-1 gathers
            with tc.tile_pool(name="att0", bufs=2) as apool:
                for t in range(NT):
                    hsl = hbuf[:, t * CLS:(t + 1) * CLS]
                    junk = apool.tile([P, CLS], f32, tag="junk")
                    rl = apool.tile([P, 1], f32, tag="rl")
                    nc.vector.tensor_tensor(out=junk[:], in0=hsl,
                                            in1=pw[:], op=ALU.mult)
                    nc.vector.tensor_reduce(out=rl[:], in_=junk[:],
                                            axis=mybir.AxisListType.X,
                                            op=ALU.add)
                    rt = apool.tile([P, 1], f32, tag="rt")
                    nc.scalar.activation(out=rt[:], in_=rl[:],
                                         func=ACTF.Sigmoid, bias=pb)
                    nc.vector.tensor_scalar(
                        out=acc[:, t * CLS:(t + 1) * CLS],
                        in0=hsl, scalar1=rt[:, 0:1], scalar2=None,
                        op0=ALU.mult)

            # ---------------- hop phase ----------------
            with tc.tile_pool(name="hop", bufs=4) as hpool, \
                 tc.tile_pool(name="hop2", bufs=2) as hpool2:
                for k in range(1, nhops + 1):
                    s_prev = float(4.0 ** (k - 1))
                    wb = None
                    for grp in groups:
                        g0 = roff[grp[0]]
                        gr = roff[grp[-1] + 1] - g0
                        gbuf = hpool.tile([P, GMAX * CLS], f16, tag="gbuf")
                        for i in range(gr):
                            r = g0 + i
                            bi = nc.gpsimd.indirect_dma_start(
                                out=gbuf[:, i * CLS:(i + 1) * CLS],
                                out_offset=None,
                                in_=ag[k - 1][:],
                                in_offset=bass.IndirectOffsetOnAxis(
                                    ap=idxt[:, r:r + 1], axis=0),
                            )
                            # alternate SWDGE queues so descriptor generation
                            # for consecutive rounds can overlap on the Q7s
                            if i % 2 == 1:
                                bi.ins.queue = "qPoolDynamic1"
                        nc.vector.tensor_tensor(
                            out=gbuf[:, :gr * CLS], in0=gbuf[:, :gr * CLS],
                            in1=nexp[:, g0 * CLS:(g0 + gr) * CLS],
                            op=ALU.mult)
                        for t in grp:
                            o = roff[t] - g0
                            Rt = R_list[t]
                            red = hpool2.tile([P, CLS], f32, tag="red")
                            nc.vector.tensor_reduce(
                                out=red[:],
                                in_=gbuf[:, o * CLS:(o + Rt) * CLS].rearrange(
                                    "p (q c) -> p c q", c=CLS),
                                axis=mybir.AxisListType.X, op=ALU.add)
                            junk = hpool2.tile([P, CLS], f32, tag="junk")
                            rl = hpool2.tile([P, 1], f32, tag="rl")
                            nc.vector.tensor_tensor(out=junk[:], in0=red[:],
                                                    in1=pw[:], op=ALU.mult)
                            nc.vector.tensor_reduce(out=rl[:], in_=junk[:],
                                                    axis=mybir.AxisListType.X,
                                                    op=ALU.add)
                            rt = hpool2.tile([P, 1], f32, tag="rt")
                            nc.scalar.activation(out=rt[:], in_=rl[:],
                                                 func=ACTF.Sigmoid,
                                                 scale=s_prev, bias=pb)
                            tmp = hpool2.tile([P, CLS], f32, tag="tmp")
                            nc.vector.tensor_scalar(
                                out=tmp[:], in0=red[:], scalar1=rt[:, 0:1],
                                scalar2=s_prev, op0=ALU.mult, op1=ALU.mult)
                            aslice = acc[:, t * CLS:(t + 1) * CLS]
                            nc.vector.tensor_tensor(
                                out=aslice, in0=aslice, in1=tmp[:], op=ALU.add)
                            if k < KHOPS:
                                if t % WG == 0:
                                    wb = hpool2.tile([P, WG * CLS], f16,
                                                     tag="wb")
                                wslot = t % WG
                                nc.scalar.activation(
                                    out=wb[:, wslot * CLS:(wslot + 1) * CLS],
                                    in_=red[:], func=ACTF.Copy, scale=0.25)
                                if t % WG == WG - 1 or t == NT - 1:
                                    tw0 = (t // WG) * WG
                                    gw = t - tw0 + 1
                                    dst_ap = comm[k][tw0 * P:(tw0 + gw) * P, :]\
                                        .rearrange("(g p) c -> p g c", p=P)
                                    nc.sync.dma_start(
                                        out=dst_ap,
                                        in_=wb[:, :gw * CLS].rearrange(
                                            "p (g c) -> p g c", c=CLS))
                                    if (do_ag and k < nhops
                                            and tw0 < NTH <= tw0 + gw + WG - 1
                                            and t >= NTH - 1):
                                        emit_ag_lo(k)
                            else:
                                # last hop: final log_softmax interleaved per
                                # write group so it hides under the gathers
                                if t % WG == 0:
                                    fwb = hpool2.tile([P, WG * CLS], f32,
                                                      tag="fwb")
                                fslot = t % WG
                                nmx = hpool2.tile([P, 1], f32, tag="nmx")
                                nc.vector.tensor_reduce(
                                    out=nmx[:], in_=aslice,
                                    axis=mybir.AxisListType.X,
                                    op=ALU.max, negate=True)
                                et = hpool2.tile([P, CLS], f32, tag="et")
                                ssum = hpool2.tile([P, 1], f32, tag="ssum")
                                nc.scalar.activation(out=et[:], in_=aslice,
                                                     func=ACTF.Exp,
                                                     bias=nmx[:, 0:1])
                                nc.vector.tensor_reduce(
                                    out=ssum[:], in_=et[:],
                                    axis=mybir.AxisListType.X, op=ALU.add)
                                lsum = hpool2.tile([P, 1], f32, tag="lsum")
                                nc.scalar.activation(out=lsum[:], in_=ssum[:],
                                                     func=ACTF.Ln)
                                bias2 = hpool2.tile([P, 1], f32, tag="bias2")
                                nc.vector.tensor_tensor(
                                    out=bias2[:], in0=nmx[:, 0:1],
                                    in1=lsum[:], op=ALU.subtract)
                                nc.scalar.activation(
                                    out=fwb[:, fslot * CLS:(fslot + 1) * CLS],
                                    in_=aslice, func=ACTF.Identity,
                                    bias=bias2[:, 0:1])
                                if t % WG == WG - 1 or t == NT - 1:
                                    tw0 = (t // WG) * WG
                                    gw = t - tw0 + 1
                                    dst_ap = out_d[tw0 * P:(tw0 + gw) * P, :]\
                                        .rearrange("(g p) c -> p g c", p=P)
                                    nc.sync.dma_start(
                                        out=dst_ap,
                                        in_=fwb[:, :gw * CLS].rearrange(
                                            "p (g c) -> p g c", c=CLS))
                    if do_ag and k < nhops:
                        emit_ag_hi(k)

    nc.compile()
    return nc


# ----------------------------------------------------------------------------
# compiled-runner plumbing (persistent jit via the axon PJRT path)
# ----------------------------------------------------------------------------

class _Runner:
    def __init__(self, nc, n_cores):
        self.nc = nc
        import jax
        from jax.sharding import Mesh, PartitionSpec
        from jax.experimental.shard_map import shard_map
        import concourse.mybir as mybir
        from concourse.bass2jax import (_bass_exec_p, install_neuronx_cc_hook,
                                        partition_id_tensor)
        install_neuronx_cc_hook()
        self.jax = jax
        self.n_cores = n_cores
        pname = nc.partition_id_tensor.name if nc.partition_id_tensor else None
        in_names, out_names, out_avals, zero_outs = [], [], [], []
        for alloc in nc.m.functions[0].allocations:
            if not isinstance(alloc, mybir.MemoryLocationSet):
                continue
            name = alloc.memorylocations[0].name
            if alloc.kind == "ExternalInput":
                if name != pname:
                    in_names.append(name)
            elif alloc.kind == "ExternalOutput":
                shape = tuple(alloc.tensor_shape)
                dtype = mybir.dt.np(alloc.dtype)
                out_names.append(name)
                out_avals.append(jax.core.ShapedArray(shape, dtype))
                zero_outs.append(np.zeros(shape, dtype))
        self.in_names, self.out_names = in_names, out_names
        self.zero_outs = zero_outs
        n_params = len(in_names)
        all_in = in_names + out_names
        if pname is not None:
            all_in.append(pname)

        def _body(*args):
            operands = list(args)
            if pname is not None:
                operands.append(partition_id_tensor())
            outs = _bass_exec_p.bind(
                *operands,
                out_avals=tuple(out_avals),
                in_names=tuple(all_in),
                out_names=tuple(out_names),
                lowering_input_output_aliases=(),
                sim_require_finite=False,
                sim_require_nnan=False,
                nc=nc,
            )
            return tuple(outs)

        devices = jax.devices()[:n_cores]
        mesh = Mesh(np.asarray(devices), ("core",))
        nio = n_params + len(out_names)
        self.fn = jax.jit(
            shard_map(_body, mesh=mesh,
                      in_specs=(PartitionSpec("core"),) * nio,
                      out_specs=(PartitionSpec("core"),) * len(out_names),
                      check_rep=False),
            keep_unused=True,
        )

    def run(self, in_maps):
        n = self.n_cores
        args = [
            np.concatenate([np.asarray(in_maps[c][k]) for c in range(n)], axis=0)
            for k in self.in_names
        ] + [np.concatenate([z] * n, axis=0) for z in self.zero_outs]
        outs = self.fn(*args)
        outs = [np.asarray(o) for o in outs]
        res = []
        for c in range(n):
            d = {}
            for name, o in zip(self.out_names, outs):
                per = o.shape[0] // n
                d[name] = o[c * per:(c + 1) * per]
            res.append(d)
        return res


def kernel(**inputs):
    in_maps, meta = _prep(**inputs)
    key = (meta["RT"], meta["R_list"], round(meta["pb"], 8))
    if key not in _COMPILED:
        nc = _build(list(meta["R_list"]), meta["RT"], meta["pb"])
        _COMPILED[key] = _Runner(nc, NCORES)
    runner = _COMPILED[key]
    res = runner.run(in_maps)

    out_full = np.empty((N, CLS), dtype=np.float32)
    order = meta["order"]
    core_of_rank = meta["core_of_rank"]
    pos_of_rank = meta["pos_of_rank"]
    for c in range(NCORES):
        mask = core_of_rank == c
        out_full[order[mask]] = res[c]["out"][pos_of_rank[mask]]
    return out_full
